# revision 25
# baseline (speedup 1.0000x reference)
"""Trainium2 Bass kernel for multi-head self-attention (nn_Attention), v4.

Sharding over 8 NeuronCores: core = (g, b), g = head-pair (2 heads), b = batch.

v4 = phased schedule + dual-engine exp + partial fp8 DoubleRow P.V:
- Phase 0: short garbage warmup on the PE (covers the ~7us sync-engine
  preamble dead window and starts the HAM clock-unlock counter), chunked input
  DMAs (wq, x in 4 token chunks, wo last), ACT exp table warm, all constant
  memsets on the (otherwise idle) GpSimd engine.
- Phase 1: projections upfront. q and k projections for the same token block
  are column-tiled (tile_position (0,0)/(0,64), M=64 each) so the two matmuls
  run concurrently in the PE array. V drains to fp8e4 V2 (DoubleRow pairs
  0,2,4,6 = kv tiles 0,1,4,5,8,9,12,13) and to f16 V16 (the rest).
- Phase 2: 4 attention units (head, 1024-span), 16 kv tiles each, processed in
  interleaved order so both exp engines run concurrently:
    ACT tiles {0,1,4,5,8,9,12,13}: native exp -> fp8e4, consumed by DoubleRow
      P.V (K=256 per pair, 0.5 cyc/col).
    DVE tiles {2,3,6,7,10,11,14,15}: i16(s*C1+C2) bitcast f16 Schraudolph
      trick (~1.7% rms, proven in v2), consumed by plain fp16 P.V.
  PSUM: S pipeline 3x[128,1024] (6 banks) + po [80,1024] (2 banks) = 8 banks.
- Phase 3: Y output projection (per head, unnormalized) batched at the end,
  reusing the S PSUM pool; psum->sbuf casts alternate DVE/ACT; host divides by
  the DEN row and sums heads (division commutes with the output projection).
"""

import os

import numpy as np

B, N, DIM = 2, 2048, 256
HEADS, D = 8, 64
INNER = HEADS * D
NH = 2
NT = N // 128  # 16 kv tiles
PAIRS = NT // 2
SPAN = 1024
NSP = N // SPAN
SUB = SPAN // 128
VP = 80  # padded V width (D + ones + zero pad, 16B-aligned for DoubleRow)
SCALE = D ** -0.5

# softmax shift: P~ = exp(s*SCALE - B0). Keeps fp8e4 (TRN e4m3: max 240,
# inf at bits 0x78) clear of overflow; cancels exactly in the host division.
B0 = 2.5
# f16 Schraudolph constants: i16(s*C1 + C2) bitcast f16 ~= exp(s*SCALE - B0)
C1_16 = float(SCALE * np.log2(np.e) * 1024.0)
C2_16 = float(15.0 * 1024.0 - 0.045 * 1024.0 - B0 * np.log2(np.e) * 1024.0)

# kv-tile -> exp engine: ACT handles DoubleRow-paired tiles, DVE the rest
FP8_PAIRS = tuple(
    int(x) for x in os.environ.get("KERNEL_FP8_PAIRS", "0,2,4,6").split(",") if x != ""
)
ACT_TILES = frozenset(t for p in FP8_PAIRS for t in (2 * p, 2 * p + 1))

_CACHE = {}


def _build_nc():
    import concourse.mybir as mybir
    import concourse.tile as tile
    from concourse import bacc

    f32 = mybir.dt.float32
    f16 = mybir.dt.float16
    bf16 = mybir.dt.bfloat16
    i16 = mybir.dt.int16
    fp8 = mybir.dt.float8e4
    DR = mybir.MatmulPerfMode.DoubleRow
    EXPF = mybir.ActivationFunctionType.Exp

    n_warm = int(os.environ.get("KERNEL_WARMUP", "8"))
    PV_DELAY = int(os.environ.get("KERNEL_PV_DELAY", "1"))
    Y_AHEAD = int(os.environ.get("KERNEL_Y_AHEAD", "2"))
    no_sc = bool(int(os.environ.get("KERNEL_NO_SCALAR_COPY", "0")))
    qk_coltile = bool(int(os.environ.get("KERNEL_QK_COLTILE", "1")))
    y_warm = bool(int(os.environ.get("KERNEL_Y_WARM", "1")))

    # interleaved processing order: alternate single ACT/DVE tiles so the two
    # exp streams run concurrently AND the 3-slot S-PSUM ring is recycled by
    # alternating engines (plain 0..15 order makes each engine wait on a slot
    # held by its own previous pair -> round-robin stall, ~3.5us/unit slower)
    dve_tiles = sorted(set(range(NT)) - ACT_TILES)
    act_tiles = sorted(ACT_TILES)
    order = []
    for i in range(8):
        if i < len(act_tiles):
            order.append(act_tiles[i])
        if i < len(dve_tiles):
            order.append(dve_tiles[i])

    nc = bacc.Bacc("TRN2", num_devices=8)
    xT16 = nc.dram_tensor("xT16", [128, 2, N], f16, kind="ExternalInput")
    wq16 = nc.dram_tensor("wq16", [128, 2, NH * 192], f16, kind="ExternalInput")
    wo16 = nc.dram_tensor("wo16", [D, NH, DIM], f16, kind="ExternalInput")
    YH = nc.dram_tensor("YH", [NH, N, DIM], f16, kind="ExternalOutput")
    DEN = nc.dram_tensor("DEN", [NH, NSP, 1, SPAN], f16, kind="ExternalOutput")

    with tile.TileContext(nc) as tc:
        with (
            tc.tile_pool(name="const", bufs=1) as const,
            tc.tile_pool(name="pex8", bufs=4) as pex8p,
            tc.tile_pool(name="pex16", bufs=6) as pex16p,
            tc.tile_pool(name="otp", bufs=4) as otp,
            tc.tile_pool(name="y16", bufs=2) as y16p,
            tc.tile_pool(name="ps", bufs=3, space="PSUM") as ps,
            tc.tile_pool(name="po", bufs=1, space="PSUM") as po,
        ):
            # ---- input DMAs (sync queue; every DMA-capable queue is blocked
            # by the framework preamble until ~6-7us anyway) -----------------
            wq_sb = const.tile([128, 2, NH * 192], f16)
            nc.sync.dma_start(wq_sb, wq16[:])
            xT_sb = const.tile([128, 2, N], f16)
            for c in range(4):
                nc.sync.dma_start(
                    xT_sb[:, :, c * 512 : (c + 1) * 512],
                    xT16[:, :, c * 512 : (c + 1) * 512],
                )
            wo_sb = const.tile([D, NH, DIM], f16)
            nc.sync.dma_start(wo_sb, wo16[:])

            # ---- all constant memsets on GpSimd: it is otherwise idle and
            # its queue unblocks earliest (~5.8us), keeping DVE/ACT free for
            # the projection drains --------------------------------------
            g16 = const.tile([128, 512], bf16)
            nc.gpsimd.memset(g16, 0.5)
            V16 = const.tile([128, NH, NT, D + 1], f16)
            nc.gpsimd.memset(V16[:, :, :, D : D + 1], 1.0)
            V2 = const.tile([128, NH, PAIRS, 2, VP], fp8)
            nc.gpsimd.memset(V2[:, :, :, :, D : D + 1], 1.0)
            nc.gpsimd.memset(V2[:, :, :, :, D + 1 : VP], 0.0)
            V2r = V2.rearrange("p h pr i v -> p h (pr i) v")
            nbias = const.tile([128, 1], f32)
            nc.gpsimd.memset(nbias, -B0)
            warm = const.tile([64, 4], f32, name="actwarm")
            nc.gpsimd.memset(warm, 0.0)
            qT = const.tile([D, NH, N], f16)
            kT = const.tile([D, NH, N], f16)

            # ---- PE warmup on garbage: keeps the PE busy until the first
            # inputs land (~8.5us) and starts the HAM clock-unlock counter ---
            for _ in range(n_warm):
                pw = ps.tile([128, 512], f32, tag="S", name="pwarm")
                nc.tensor.matmul(pw, g16[:, 0:128], g16, start=True, stop=True)

            # ACT exp table warm while DMAs run
            nc.scalar.activation(warm, warm, EXPF)

            # ---- phase 1: projections --------------------------------------
            eng = [0]

            def drain(dst, src):
                if no_sc or eng[0] % 2 == 0:
                    nc.vector.tensor_copy(dst, src)
                else:
                    nc.scalar.copy(dst, src)
                eng[0] += 1

            def bridge(n):
                # dependency-free garbage matmuls: keep the PE busy-streak
                # alive across phase boundaries so HAM stays at full clock
                # (a single >3us idle gap re-throttles to 1.2GHz, and a cold
                # attention phase can stay cold for 50+us)
                pwb = ps.tile([128, 512], f32, tag="S", name="pwb")
                for _ in range(n):
                    nc.tensor.matmul(pwb, g16[:, 0:128], g16, start=True, stop=True)

            # alternate projection psum tiles between the ps pool (3 bufs)
            # and the otherwise-idle po pool for a 4th slot: keeps the PE
            # ahead of the drains so HAM sees a continuous busy streak
            def ptile(shape, name):
                return ps.tile(shape, f32, tag="S", name=name)

            def emit_qk2(hh, blk):
                # q and k rows are adjacent in wq: one [128,128] stationary
                # computes both projections in a single matmul
                pp = ptile([128, 512], "pp")
                for c in range(2):
                    nc.tensor.matmul(
                        pp,
                        wq_sb[:, c, hh * 192 : hh * 192 + 2 * D],
                        xT_sb[:, c, blk * 512 : (blk + 1) * 512],
                        start=(c == 0),
                        stop=(c == 1),
                    )
                drain(qT[:, hh, blk * 512 : (blk + 1) * 512], pp[0:D, :])
                drain(kT[:, hh, blk * 512 : (blk + 1) * 512], pp[D : 2 * D, :])

            # wq v-slices of both heads as one strided AP: [128, c, hh, 64]
            wqv = wq_sb.rearrange("p c (h m) -> p c h m", h=NH)[:, :, :, 2 * D : 3 * D]

            def emit_v2(p):
                # both kv tiles of pair p -> one psum tile, one drain
                pvb2 = ptile([128, 2, NH, D], "pvb2")
                for i in range(2):
                    for c in range(2):
                        nc.tensor.matmul(
                            pvb2[:, i, :, :],
                            xT_sb[:, c, (2 * p + i) * 128 : (2 * p + i + 1) * 128],
                            wqv[:, c, :, :],
                            start=(c == 0),
                            stop=(c == 1),
                        )
                src = pvb2.rearrange("t i h d -> t h i d")
                if p in FP8_PAIRS:
                    drain(V2[:, :, p, :, 0:D], src)
                else:
                    drain(
                        V16.rearrange("t h n d -> t h n d")[:, :, 2 * p : 2 * p + 2, 0:D],
                        src,
                    )

            # minimal upfront projections: unit (0,0) needs q blocks 0-1,
            # k block 0 and V pair 0; the rest stream in as background work
            # inside unit 0's tile loop (one chunk per kv tile)
            emit_qk2(0, 0)
            emit_qk2(0, 1)
            emit_v2(0)
            background = [
                lambda: emit_v2(1),
                lambda: emit_qk2(0, 2),
                lambda: emit_v2(2),
                lambda: emit_v2(3),
                lambda: emit_qk2(0, 3),
                lambda: emit_v2(4),
                lambda: emit_v2(5),
                lambda: emit_qk2(1, 0),
                lambda: emit_v2(6),
                lambda: emit_v2(7),
                lambda: emit_qk2(1, 1),
                lambda: emit_qk2(1, 2),
                lambda: emit_qk2(1, 3),
            ]
            bridge(6)

            # ---- phase 2: attention units ----------------------------------
            units = [(hh, s) for hh in range(NH) for s in range(NSP)]
            ots = []

            for hh, s in units:
                po_t = po.tile([VP, SPAN], f32, tag="O", name="po_t")
                pS_t = {}

                def emit_st(t, hh=hh, s=s, pS_t=pS_t):
                    pS = ps.tile([128, SPAN], f32, tag="S", name="pS")
                    pS_t[t] = pS
                    for half in range(2):
                        nc.tensor.matmul(
                            pS[:, half * 512 : (half + 1) * 512],
                            kT[:, hh, t * 128 : (t + 1) * 128],
                            qT[
                                :,
                                hh,
                                s * SPAN + half * 512 : s * SPAN + (half + 1) * 512,
                            ],
                            start=True,
                            stop=True,
                        )

                emit_st(order[0])
                emit_st(order[1])
                pex8_cur = [None]
                n_pv = [0]

                def emit_pv(lhsT, rhs, perf_mode=None, ntiles=1):
                    for half in range(2):
                        nc.tensor.matmul(
                            po_t[0 : (VP if perf_mode else D + 1), half * 512 : (half + 1) * 512],
                            lhsT,
                            rhs[:, half * 512 : (half + 1) * 512]
                            if perf_mode is None
                            else rhs[:, :, half * 512 : (half + 1) * 512],
                            start=(n_pv[0] == 0),
                            stop=(n_pv[0] + ntiles == NT),
                            perf_mode=perf_mode,
                        )
                    n_pv[0] += ntiles

                # PVs are flushed one iteration late: a PV emitted right after
                # its exp would sit at the head of the PE FIFO waiting ~1.2us
                # for the exp, blocking the next (ready) S fill behind it
                pending_pv = []
                for idx, t in enumerate(order):
                    if idx + 2 < NT:
                        emit_st(order[idx + 2])
                    if background:
                        background.pop(0)()
                    if len(pending_pv) >= PV_DELAY:
                        pending_pv.pop(0)()
                    if t in ACT_TILES:
                        p, i = t // 2, t % 2
                        if i == 0:
                            pex8_cur[0] = pex8p.tile([128, 2, SPAN], fp8, name="pex8")
                        nc.scalar.activation(
                            pex8_cur[0][:, i, :],
                            pS_t.pop(t),
                            EXPF,
                            scale=SCALE,
                            bias=nbias,
                        )
                        if i == 1:
                            # one DoubleRow matmul pair consumes two kv tiles
                            pex8 = pex8_cur[0]
                            pending_pv.append(
                                lambda p=p, pex8=pex8: emit_pv(
                                    V2[:, hh, p, :, :], pex8, DR, ntiles=2
                                )
                            )
                    else:
                        pex16 = pex16p.tile([128, SPAN], i16, name="pex16")
                        nc.vector.tensor_scalar(
                            pex16,
                            pS_t.pop(t),
                            C1_16,
                            C2_16,
                            mybir.AluOpType.mult,
                            mybir.AluOpType.add,
                        )
                        pending_pv.append(
                            lambda t=t, pex16=pex16: emit_pv(
                                V16[:, hh, t, :], pex16.bitcast(f16)
                            )
                        )
                while pending_pv:
                    pending_pv.pop(0)()
                # output head: O^T rows 0..63 + denominator row 64, fp16.
                # Both halves on ACT: balances ACT (8x1.10us exp) against
                # DVE (8x1.22us exp) within the unit.
                ot_t = otp.tile([D + 1, SPAN], f16, name="ot_t")
                if no_sc:
                    nc.vector.tensor_copy(ot_t[:, 0 : SPAN // 2], po_t[0 : D + 1, 0 : SPAN // 2])
                    nc.vector.tensor_copy(ot_t[:, SPAN // 2 :], po_t[0 : D + 1, SPAN // 2 :])
                else:
                    nc.scalar.copy(ot_t[:, 0 : SPAN // 2], po_t[0 : D + 1, 0 : SPAN // 2])
                    nc.scalar.copy(ot_t[:, SPAN // 2 :], po_t[0 : D + 1, SPAN // 2 :])
                nc.sync.dma_start(DEN[hh, s], ot_t[D : D + 1, :])
                ots.append((hh, s, ot_t))

            # ---- phase 3: output projection (per head, unnormalized) -------
            # matmuls run Y_AHEAD of the casts (PE stays dense, no HAM
            # re-throttle); casts alternate DVE/ACT; DMA per half-span
            pyts = {}
            ysps = {}
            ycount = [0]

            def y_mm(g):
                hh, s, ot_t = ots[g // SUB]
                j = g % SUB
                if j == 0:
                    ysps[g // SUB] = y16p.tile([128, SUB, DIM], f16, name="ysp")
                pyt = ps.tile([128, DIM], f32, tag="S", name="pyt")
                pyts[g] = pyt
                nc.tensor.matmul(
                    pyt,
                    ot_t[0:D, j * 128 : (j + 1) * 128],
                    wo_sb[:, hh, :],
                    start=True,
                    stop=True,
                )

            def y_cast(g):
                hh, s, ot_t = ots[g // SUB]
                j = g % SUB
                ysp = ysps[g // SUB]
                if no_sc or g % 2 == 0:
                    nc.vector.tensor_copy(ysp[:, j, :], pyts.pop(g))
                else:
                    nc.scalar.copy(ysp[:, j, :], pyts.pop(g))
                if j + 1 in (SUB // 2, SUB):
                    j0 = 0 if j + 1 == SUB // 2 else SUB // 2
                    nc.sync.dma_start(
                        YH[
                            hh,
                            s * SPAN + j0 * 128 : s * SPAN + (j + 1) * 128,
                            :,
                        ].rearrange("(j p) m -> p j m", p=128),
                        ysp[:, j0 : j + 1, :],
                    )

            NY = len(ots) * SUB
            for g in range(NY):
                y_mm(g)
                if g >= Y_AHEAD:
                    y_cast(g - Y_AHEAD)
            for g in range(NY - Y_AHEAD, NY):
                y_cast(g)
    nc.compile()
    return nc


def get_nc():
    key = (
        "nc_v4",
        os.environ.get("KERNEL_FP8_PAIRS", "0,2,4,6"),
        os.environ.get("KERNEL_WARMUP", "12"),
        os.environ.get("KERNEL_QK_COLTILE", "1"),
        os.environ.get("KERNEL_NO_SCALAR_COPY", "0"),
        os.environ.get("KERNEL_PV_DELAY", "1"),
        os.environ.get("KERNEL_Y_AHEAD", "2"),
    )
    if key not in _CACHE:
        _CACHE[key] = _build_nc()
    return _CACHE[key]


def _to_f16(a):
    return np.ascontiguousarray(a.astype(np.float16))


def make_in_maps(x, w_qkv, w_out):
    x = np.asarray(x, dtype=np.float32)
    w_qkv = np.asarray(w_qkv, dtype=np.float32)
    w_out = np.asarray(w_out, dtype=np.float32)
    in_maps = []
    for core in range(8):
        g, b = core % 4, core // 4
        xT = x[b].T  # [256, 2048]
        xT16 = _to_f16(xT.reshape(2, 128, N).transpose(1, 0, 2))
        wslice = w_qkv[g * 384 : (g + 1) * 384]  # [384, 256]
        wq16 = _to_f16(wslice.T.reshape(2, 128, NH * 192).transpose(1, 0, 2))
        wo16 = np.ascontiguousarray(
            np.stack(
                [
                    w_out[:, g * 128 + h * D : g * 128 + (h + 1) * D].T
                    for h in range(NH)
                ],
                axis=1,
            ).astype(np.float16)
        )
        in_maps.append({"xT16": xT16, "wq16": wq16, "wo16": wo16})
    return in_maps


def gather(results, b_out):
    y = np.zeros((B, N, DIM), np.float32)
    for core in range(8):
        g, b = core % 4, core // 4
        yh = results[core]["YH"].astype(np.float32)  # [NH, N, DIM]
        den = results[core]["DEN"].astype(np.float32).reshape(NH, N)
        y[b] += yh[0] / den[0][:, None]
        y[b] += yh[1] / den[1][:, None]
    y += np.asarray(b_out, dtype=np.float32)[None, None, :]
    return y


def kernel(x, mask, w_qkv, w_out, b_out):
    if not os.environ.get("KERNEL_TRACE"):
        os.environ.setdefault("BASS_NEVER_TRACE", "1")
    from concourse.bass_utils import run_bass_kernel_spmd

    nc = get_nc()
    in_maps = make_in_maps(x, w_qkv, w_out)
    br = run_bass_kernel_spmd(nc, in_maps, core_ids=list(range(8)))
    _CACHE["last_br"] = br
    return gather(br.results, b_out)


def run_traced(x, mask, w_qkv, w_out, b_out, tmpdir, trace_cores=(0,)):
    from concourse.bass_utils import run_bass_kernel_spmd

    nc = get_nc()
    in_maps = make_in_maps(x, w_qkv, w_out)
    br = run_bass_kernel_spmd(
        nc,
        in_maps,
        core_ids=list(range(8)),
        trace=True,
        tmpdir=tmpdir,
        trace_cores=list(trace_cores),
    )
    return gather(br.results, b_out), br
